# revision 25
# baseline (speedup 1.0000x reference)
"""Multi-head attention (RoPE, causal) Bass kernel for 8 TRN2 NeuronCores.

Problem: x[2,2048,1024], 16 heads x 64 dim, causal mask, RoPE, f32.

Sharding: batch x head-group. Core c handles batch c//4 and the 4 heads
[4*(c%4), 4*(c%4)+4). Each core computes q/k/v projections for its head
slice, RoPE, causal attention, and a partial output projection against its
rows of Wo.T. The host sums the 4 partials per batch (the "all-reduce" of
the row-split output projection is done on the host during unsharding).

Device layout notes:
- All inputs stream in bf16 (f32 PSUM accumulation everywhere); the output
  partials return in bf16 and are upcast + summed on the host.
- x is passed pre-transposed per batch: xT [1024, 2048] so it can stream as
  the matmul moving operand.
- Wq/Wk rows are permuted per head to [e0..e15, o0..o15, e16..e31, o16..o31]
  (e/o = even/odd RoPE pair lanes) so the RoPE rotate-half becomes a
  16<->16 swap inside each 32-partition group (one DVE stream_shuffle).
- Causal masking of the diagonal 128-blocks is a post-exp DVE multiply with
  a 0/1 upper-triangular tile (cheaper than a -inf accumulate matmul).
- PV runs q-major: out[q,65] = probs[k,q].T @ [v|1][k,65] accumulated over
  k-chunks. The appended ones column makes the softmax denominator land in
  column 64 -- one value per PSUM partition -- so normalization is a cheap
  strided reciprocal + per-partition multiply. The normalized [q, dv] tiles
  are transposed back to [dv, q] on the PE for the output projection.
- Scheduling: the s-block-1 projections, PE transposes and output
  projection are emitted as filler closures woven between attention groups,
  and PV matmuls trail their scores by two groups, so the tensor queue
  always holds ready work and never idles waiting on the scalar engine's
  EXP (the PE p-state drops ~2x for 3us after any idle gap).
"""

import numpy as np
import ml_dtypes

import concourse.bass as bass
import concourse.mybir as mybir
import concourse.tile as tile
from concourse import bacc
from concourse.bass_utils import run_bass_kernel_spmd

F32 = mybir.dt.float32
BF16 = mybir.dt.bfloat16

B, S, D = 2, 2048, 1024
H, HD = 16, 64
NCORES = 8
HPC = 4          # heads per core
DQ = HPC * HD    # 256 projected dims per core
THETA = 10000.0

_cached = {}


def build_nc():
    """Build the single-core Bass graph (same NEFF runs SPMD on all 8)."""
    nc = bacc.Bacc("TRN2", target_bir_lowering=False)

    xt_d = nc.dram_tensor("xt", [D, S], BF16, kind="ExternalInput")
    wq_d = nc.dram_tensor("wq", [D, DQ], BF16, kind="ExternalInput")
    wk_d = nc.dram_tensor("wk", [D, DQ], BF16, kind="ExternalInput")
    wv_d = nc.dram_tensor("wv", [D, DQ], BF16, kind="ExternalInput")
    wo_d = nc.dram_tensor("wo", [DQ, D], BF16, kind="ExternalInput")
    cs_d = nc.dram_tensor("cs", [128, 2 * S], BF16, kind="ExternalInput")
    ti_d = nc.dram_tensor("ti", [128, 256], BF16, kind="ExternalInput")
    out_d = nc.dram_tensor("out", [S, D], BF16, kind="ExternalOutput")

    Exp = mybir.ActivationFunctionType.Exp
    SHUF = [(i + 16) % 32 for i in range(32)]  # 16<->16 swap per 32-group

    with tile.TileContext(nc) as tc:
        with (
            tc.tile_pool(name="consts", bufs=1) as consts,
            tc.tile_pool(name="big", bufs=8) as bigp,
            tc.tile_pool(name="qk", bufs=1) as qkp,
            tc.tile_pool(name="vsb", bufs=1) as vp,
            tc.tile_pool(name="rope", bufs=4) as ropep,
            tc.tile_pool(name="probs", bufs=6) as probsp,
            tc.tile_pool(name="otq", bufs=2) as otqp,
            tc.tile_pool(name="small", bufs=3) as smallp,
            tc.tile_pool(name="ps", bufs=2, space="PSUM") as psp,
            tc.tile_pool(name="pvq", bufs=1, space="PSUM") as pvqp,
            tc.tile_pool(name="po", bufs=2, space="PSUM") as pop,
        ):
            # ---- weights + xT (staged: first 512 cols ahead, k-proj first) ----
            wk_sb = consts.tile([128, 8, DQ], BF16, tag="wk")
            nc.sync.dma_start(out=wk_sb, in_=wk_d.rearrange("(k p) m -> p k m", p=128))

            xt = []
            for k in range(8):
                t = bigp.tile([128, S], BF16, tag="big", name=f"xt{k}")
                xt.append(t)
            for c0, c1 in ((0, 512), (512, 1024)):
                for k in range(8):
                    nc.sync.dma_start(out=xt[k][:, c0:c1],
                                      in_=xt_d[128 * k:128 * (k + 1), c0:c1])

            wv_sb = consts.tile([128, 8, DQ], BF16, tag="wv")
            nc.sync.dma_start(out=wv_sb, in_=wv_d.rearrange("(k p) m -> p k m", p=128))
            wq_sb = consts.tile([128, 8, DQ], BF16, tag="wq")
            nc.sync.dma_start(out=wq_sb, in_=wq_d.rearrange("(k p) m -> p k m", p=128))

            cs_sb = consts.tile([128, 2, S], BF16, tag="cs")
            nc.sync.dma_start(out=cs_sb, in_=cs_d.rearrange("p (a b) -> p a b", a=2))
            cos_sb = cs_sb[:, 0, :]
            sin_sb = cs_sb[:, 1, :]
            ti_sb = consts.tile([128, 2, 128], BF16, tag="ti")
            nc.sync.dma_start(out=ti_sb, in_=ti_d.rearrange("p (a b) -> p a b", a=2))
            mask_sb = ti_sb[:, 0, :]   # 0/1 upper-tri (k <= q) causal mask
            ident_sb = ti_sb[:, 1, :]
            wo_sb = consts.tile([128, 2, D], BF16, tag="wo")
            nc.sync.dma_start(out=wo_sb, in_=wo_d.rearrange("(k p) m -> p k m", p=128))

            # q/k destination tiles: [pair][128 rows = 2 heads x 64, S]
            qt = [qkp.tile([128, S], BF16, tag=f"qt{p}", name=f"qt{p}") for p in range(2)]
            kt = [qkp.tile([128, S], BF16, tag=f"kt{p}", name=f"kt{p}") for p in range(2)]
            # v tiles: per s-chunk [128, 4*65] ([v_h | 1] per head)
            vsb = [vp.tile([128, 4 * 65], BF16, tag=f"v{i}", name=f"v{i}") for i in range(16)]
            # attention output (pre out-proj): [pair][128 = 2 heads x 64 dv, S]
            ot = [qkp.tile([128, S], BF16, tag=f"ot{p}", name=f"ot{p}") for p in range(2)]

            # ---- projection emitters (one unit = one 8-deep matmul group) ----
            def rope_unit(w_sb, dst, n, m, half, on_act):
                hcol = slice(1024 * n + 512 * half, 1024 * n + 512 * half + 512)
                mcol = slice(128 * m, 128 * (m + 1))
                ps = psp.tile([128, 512], F32, tag="ps", name="ps")
                for k in range(8):
                    nc.tensor.matmul(
                        ps,
                        lhsT=w_sb[:, k, mcol],
                        rhs=xt[k][:, hcol],
                        start=(k == 0),
                        stop=(k == 7),
                    )
                # RoPE: dst = raw*cos + shuf(raw)*sin
                raw = ropep.tile([128, 512], BF16, tag="raw")
                (nc.scalar.copy if on_act else nc.vector.tensor_copy)(raw, ps)
                rot = ropep.tile([128, 512], BF16, tag="rot")
                nc.vector.stream_shuffle(rot, raw, SHUF)
                t1 = ropep.tile([128, 512], BF16, tag="rot", name="t1")
                nc.vector.tensor_mul(t1, raw, cos_sb[:, hcol])
                t2 = ropep.tile([128, 512], BF16, tag="t2")
                nc.vector.tensor_mul(t2, rot, sin_sb[:, hcol])
                nc.vector.tensor_add(dst[m][:, hcol], t1, t2)

            def v_unit(n, g, on_act):
                # Two s-chunks share one psum bank as one accumulation group
                # (start on the first chunk's k=0, the second chunk's k=0
                # overwrites its pending-zero half, stop on its k=7).
                psv = psp.tile([128, 2, 512], F32, tag="ps", name="psv")
                for sub in range(4):
                    i = 8 * n + 4 * g + sub
                    scol = slice(128 * i, 128 * (i + 1))
                    half = slice(256 * (sub % 2), 256 * (sub % 2) + 256)
                    for k in range(8):
                        nc.tensor.matmul(
                            psv[:, sub // 2, half],
                            lhsT=xt[k][:, scol],
                            rhs=wv_sb[:, k, :],
                            start=(sub % 2 == 0 and k == 0),
                            stop=(sub % 2 == 1 and k == 7),
                        )
                for sub in range(4):
                    i = 8 * n + 4 * g + sub
                    half = slice(256 * (sub % 2), 256 * (sub % 2) + 256)
                    nc.vector.memset(
                        vsb[i].rearrange("p (h c) -> p h c", c=65)[:, :, 64],
                        1.0,
                    )
                    (nc.scalar.copy if on_act else nc.vector.tensor_copy)(
                        vsb[i].rearrange("p (h c) -> p h c", c=65)[:, :, 0:64],
                        psv[:, sub // 2, half].rearrange("p (h c) -> p h c", c=64),
                    )

            # ---- phase A: k/v/q projections for s-block 0 (pure tensor) ----
            for m in range(2):
                for half in range(2):
                    rope_unit(wk_sb, kt, 0, m, half, on_act=True)
            for g in range(2):
                v_unit(0, g, on_act=True)
            for m in range(2):
                for half in range(2):
                    rope_unit(wq_sb, qt, 0, m, half, on_act=True)
            for k in range(8):
                nc.sync.dma_start(out=xt[k][:, 1024:2048],
                                  in_=xt_d[128 * k:128 * (k + 1), 1024:2048])

            # ---- filler queue: work woven between attention groups so the
            # tensor engine never drains (s-block-1 projections first, then
            # transposes + output projections appended as they become legal).
            filler = []

            def tick(k=1):
                for _ in range(k):
                    if filler:
                        filler.pop(0)()

            for m in range(2):
                for half in range(2):
                    filler.append(
                        lambda m=m, half=half:
                        rope_unit(wk_sb, kt, 1, m, half, on_act=True))
            for g in range(2):
                filler.append(lambda g=g: v_unit(1, g, on_act=True))
            for m in range(2):
                for half in range(2):
                    filler.append(
                        lambda m=m, half=half:
                        rope_unit(wq_sb, qt, 1, m, half, on_act=True))

            def mk_transpose(j, p, otq):
                jcol = slice(512 * j, 512 * (j + 1))

                def go():
                    tp = psp.tile([128, 4, 128], BF16, tag="ps", name="tp")
                    for qc in range(4):
                        nc.tensor.matmul(
                            tp[:, qc, :],
                            lhsT=otq[:, qc, :],
                            rhs=ident_sb,
                            is_transpose=True,
                            start=(qc == 0),
                            stop=(qc == 3),
                            skip_group_check=True,
                        )
                    nc.vector.tensor_copy(ot[p][:, jcol],
                                          tp.rearrange("p a b -> p (a b)"))
                return go

            def mk_po(m):
                def go():
                    mcol = slice(128 * m, 128 * (m + 1))
                    posb = bigp.tile([128, D], BF16, tag="big", name="posb")
                    for d in range(2):
                        po = pop.tile([128, 512], F32, tag="po", name="po")
                        for pp in range(2):
                            nc.tensor.matmul(
                                po,
                                lhsT=ot[pp][:, mcol],
                                rhs=wo_sb[:, pp, 512 * d:512 * (d + 1)],
                                start=(pp == 0),
                                stop=(pp == 1),
                            )
                        eng = nc.vector.tensor_copy if d == 0 else nc.scalar.copy
                        eng(posb[:, 512 * d:512 * (d + 1)], po)
                    nc.sync.dma_start(out=out_d[mcol, :], in_=posb)
                return go

            # ---- phase B: attention, PV lagging its scores by two groups ----
            for j in range(4):
                if j > 0:
                    for m in range(4 * (j - 1), 4 * j):
                        filler.append(mk_po(m))
                for p in range(2):
                    pvq = [
                        pvqp.tile([128, 4, 65], F32, tag=f"pvq{h}",
                                  name=f"pvq{h}")
                        for h in range(2)
                    ]
                    started = [False, False]
                    pvdefer = []

                    def mk_pv(i, probs):
                        r = i - 4 * j

                        def go():
                            for h in range(2):
                                hh = 2 * p + h
                                for qc in range(3, max(r, 0) - 1, -1):
                                    nc.tensor.matmul(
                                        pvq[h][:, qc, :],
                                        lhsT=probs[:, h,
                                                   128 * qc:128 * (qc + 1)],
                                        rhs=vsb[i][:, 65 * hh:65 * hh + 65],
                                        start=(not started[h]),
                                        stop=(i == 4 * j + qc),
                                        skip_group_check=True,
                                    )
                                    started[h] = True
                        return go

                    for i in range(4 * j + 4):
                        r = i - 4 * j
                        loc = max(0, 128 * r)
                        icol = slice(128 * i, 128 * (i + 1))
                        probs = probsp.tile([128, 2, 512], BF16, tag="probs")
                        sc = psp.tile([128, 2, 512], F32, tag="ps", name="sc")
                        for h in range(2):
                            rows = slice(64 * h, 64 * (h + 1))
                            nc.tensor.matmul(
                                sc[:, h, loc:512],
                                lhsT=kt[p][rows, icol],
                                rhs=qt[p][rows, 512 * j + loc:512 * (j + 1)],
                                start=True,
                                stop=True,
                            )
                        nc.scalar.activation(
                            probs[:, :, loc:512], sc[:, :, loc:512], Exp
                        )
                        if r >= 0:
                            for h in range(2):
                                nc.vector.tensor_mul(
                                    probs[:, h, loc:loc + 128],
                                    probs[:, h, loc:loc + 128],
                                    mask_sb,
                                )
                        pvdefer.append(mk_pv(i, probs))
                        if len(pvdefer) > 3:
                            pvdefer.pop(0)()
                        if i % 3 == 0 if j < 2 else True:
                            tick()
                    while pvdefer:
                        pvdefer.pop(0)()
                        tick()
                    # normalization (DVE): otq[:, qc, 64h:64h+64] =
                    #   pvq[h][:, qc, 0:64] / den  (den = column 64)
                    otq = otqp.tile([128, 4, 128], BF16, tag="otq", name="otq")
                    for h in range(2):
                        rd = smallp.tile([128, 4], F32, tag="rd", name="rd")
                        nc.vector.reciprocal(rd, pvq[h][:, :, 64])
                        for qc in range(4):
                            nc.vector.tensor_scalar_mul(
                                otq[:, qc, 64 * h:64 * (h + 1)],
                                pvq[h][:, qc, 0:64],
                                rd[:, qc:qc + 1],
                            )
                    filler.append(mk_transpose(j, p, otq))
            while filler:
                tick()
            for m in range(12, 16):
                mk_po(m)()

    nc.compile()
    return nc


def _host_inputs(x, Wq, Wk, Wv, Wo, token_positions):
    """Build per-core input maps (all host-side numpy prep)."""
    bf = ml_dtypes.bfloat16
    x = np.asarray(x, dtype=np.float32)
    Wq = np.asarray(Wq, dtype=np.float32)
    Wk = np.asarray(Wk, dtype=np.float32)
    Wv = np.asarray(Wv, dtype=np.float32)
    Wo = np.asarray(Wo, dtype=np.float32)
    pos = np.asarray(token_positions).astype(np.float64)

    # RoPE tables in the permuted-lane layout (16-lane e/o blocks).
    idx = np.arange(0, HD, 2, dtype=np.float64) / HD
    freqs = 1.0 / THETA ** idx                      # [32]
    ang = pos[:, None] * freqs[None, :]             # [S, 32]
    c, s = np.cos(ang).T, np.sin(ang).T             # [32, S]
    c64 = np.concatenate([c[0:16], c[0:16], c[16:32], c[16:32]], 0)
    s64 = np.concatenate([-s[0:16], s[0:16], -s[16:32], s[16:32]], 0)
    cos128 = np.concatenate([c64, c64], 0).astype(np.float32)
    sin128 = np.concatenate([s64, s64], 0).astype(np.float32)
    cs128 = np.concatenate([cos128, sin128], 1).astype(bf)  # [128, 2S]

    # 0/1 upper-triangular causal mask (valid iff k <= q) + identity
    mask01 = np.triu(np.ones((128, 128), dtype=np.float32))
    ident = np.eye(128, dtype=np.float32)
    ti = np.concatenate([mask01, ident], 1).astype(bf)      # [128, 256]

    # per-head row permutation: [e0..e15, o0..o15, e16..e31, o16..o31]
    perm64 = np.concatenate([
        np.arange(0, 32, 2), np.arange(1, 32, 2),
        np.arange(32, 64, 2), np.arange(33, 64, 2),
    ])

    xts = [np.ascontiguousarray(x[b].T).astype(bf) for b in range(B)]

    in_maps = []
    for core in range(NCORES):
        b = core // 4
        heads = [4 * (core % 4) + hh for hh in range(HPC)]
        qk_rows = np.concatenate([g * HD + perm64 for g in heads])
        v_rows = np.concatenate([np.arange(g * HD, (g + 1) * HD) for g in heads])
        in_maps.append({
            "xt": xts[b],
            "wq": (np.ascontiguousarray(Wq[qk_rows, :].T) / np.sqrt(HD)).astype(bf),
            "wk": np.ascontiguousarray(Wk[qk_rows, :].T).astype(bf),
            "wv": np.ascontiguousarray(Wv[v_rows, :].T).astype(bf),
            "wo": np.ascontiguousarray(Wo[:, v_rows].T).astype(bf),
            "cs": cs128,
            "ti": ti,
        })
    return in_maps


def _ensure_ntff_hook():
    """Register the axon NTFF profile hook if the image's antenv lacks it."""
    import sys, types
    try:
        import antenv.axon_hooks  # noqa: F401
        return
    except ImportError:
        pass
    try:
        from trn_agent_boot.trn_boot import _ntff_profile_via_ctypes
        hook = _ntff_profile_via_ctypes("/opt/axon/libaxon_pjrt.so")
    except Exception:
        return
    mod = types.ModuleType("antenv.axon_hooks")
    mod.get_axon_ntff_profile_hook = lambda: hook
    mod.set_axon_ntff_profile_hook = lambda h: None
    sys.modules["antenv.axon_hooks"] = mod


def run(inputs, trace=False):
    """Run the SPMD kernel; returns (full_output, BassKernelResults)."""
    if trace:
        _ensure_ntff_hook()
    if "nc" not in _cached:
        _cached["nc"] = build_nc()
    nc = _cached["nc"]
    in_maps = _host_inputs(
        inputs["x"], inputs["Wq"], inputs["Wk"], inputs["Wv"], inputs["Wo"],
        inputs["token_positions"],
    )
    res = run_bass_kernel_spmd(nc, in_maps, core_ids=list(range(NCORES)),
                               trace=trace)
    out = np.zeros((B, S, D), dtype=np.float32)
    for core in range(NCORES):
        out[core // 4] += res.results[core]["out"].astype(np.float32)
    return out, res


def kernel(**inputs) -> np.ndarray:
    out, _ = run(inputs, trace=False)
    return out


# revision 26
# speedup vs baseline: 1.0298x; 1.0298x over previous
"""Multi-head attention (RoPE, causal) Bass kernel for 8 TRN2 NeuronCores.

Problem: x[2,2048,1024], 16 heads x 64 dim, causal mask, RoPE, f32.

Sharding: batch x head-group. Core c handles batch c//4 and the 4 heads
[4*(c%4), 4*(c%4)+4). Each core computes q/k/v projections for its head
slice, RoPE, causal attention, and a partial output projection against its
rows of Wo.T. The host sums the 4 partials per batch (the "all-reduce" of
the row-split output projection is done on the host during unsharding).

Device layout notes:
- All inputs stream in bf16 (f32 PSUM accumulation everywhere); the output
  partials return in bf16 and are upcast + summed on the host.
- x is passed pre-transposed per batch: xT [1024, 2048] so it can stream as
  the matmul moving operand.
- Wq/Wk rows are permuted per head to [e0..e15, o0..o15, e16..e31, o16..o31]
  (e/o = even/odd RoPE pair lanes) so the RoPE rotate-half becomes a
  16<->16 swap inside each 32-partition group (one DVE stream_shuffle).
- Causal masking of the diagonal 128-blocks is a post-exp DVE multiply with
  a 0/1 upper-triangular tile (cheaper than a -inf accumulate matmul).
- PV runs q-major: out[q,65] = probs[k,q].T @ [v|1][k,65] accumulated over
  k-chunks. The appended ones column makes the softmax denominator land in
  column 64 -- one value per PSUM partition -- so normalization is a cheap
  strided reciprocal + per-partition multiply. The normalized [q, dv] tiles
  are transposed back to [dv, q] on the PE for the output projection.
- Scheduling: the s-block-1 projections, PE transposes and output
  projection are emitted as filler closures woven between attention groups,
  and PV matmuls trail their scores by two groups, so the tensor queue
  always holds ready work and never idles waiting on the scalar engine's
  EXP (the PE p-state drops ~2x for 3us after any idle gap).
"""

import numpy as np
import ml_dtypes

import concourse.bass as bass
import concourse.mybir as mybir
import concourse.tile as tile
from concourse import bacc
from concourse.bass_utils import run_bass_kernel_spmd

F32 = mybir.dt.float32
BF16 = mybir.dt.bfloat16

B, S, D = 2, 2048, 1024
H, HD = 16, 64
NCORES = 8
HPC = 4          # heads per core
DQ = HPC * HD    # 256 projected dims per core
THETA = 10000.0

_cached = {}


def build_nc():
    """Build the single-core Bass graph (same NEFF runs SPMD on all 8)."""
    nc = bacc.Bacc("TRN2", target_bir_lowering=False)

    xt_d = nc.dram_tensor("xt", [D, S], BF16, kind="ExternalInput")
    wq_d = nc.dram_tensor("wq", [D, DQ], BF16, kind="ExternalInput")
    wk_d = nc.dram_tensor("wk", [D, DQ], BF16, kind="ExternalInput")
    wv_d = nc.dram_tensor("wv", [D, DQ], BF16, kind="ExternalInput")
    wo_d = nc.dram_tensor("wo", [DQ, D], BF16, kind="ExternalInput")
    cs_d = nc.dram_tensor("cs", [128, 2 * S], BF16, kind="ExternalInput")
    ti_d = nc.dram_tensor("ti", [128, 256], BF16, kind="ExternalInput")
    out_d = nc.dram_tensor("out", [S, D], BF16, kind="ExternalOutput")

    Exp = mybir.ActivationFunctionType.Exp
    SHUF = [(i + 16) % 32 for i in range(32)]  # 16<->16 swap per 32-group

    with tile.TileContext(nc) as tc:
        with (
            tc.tile_pool(name="consts", bufs=1) as consts,
            tc.tile_pool(name="big", bufs=8) as bigp,
            tc.tile_pool(name="qk", bufs=1) as qkp,
            tc.tile_pool(name="vsb", bufs=1) as vp,
            tc.tile_pool(name="rope", bufs=4) as ropep,
            tc.tile_pool(name="probs", bufs=4) as probsp,
            tc.tile_pool(name="otq", bufs=2) as otqp,
            tc.tile_pool(name="small", bufs=3) as smallp,
            tc.tile_pool(name="ps", bufs=2, space="PSUM") as psp,
            tc.tile_pool(name="pvq", bufs=1, space="PSUM") as pvqp,
            tc.tile_pool(name="po", bufs=2, space="PSUM") as pop,
        ):
            # ---- weights + xT (staged: first 512 cols ahead, k-proj first) ----
            wk_sb = consts.tile([128, 8, DQ], BF16, tag="wk")
            nc.sync.dma_start(out=wk_sb, in_=wk_d.rearrange("(k p) m -> p k m", p=128))

            xt = []
            for k in range(8):
                t = bigp.tile([128, S], BF16, tag="big", name=f"xt{k}")
                xt.append(t)
            for c0, c1 in ((0, 512), (512, 1024)):
                for k in range(8):
                    nc.sync.dma_start(out=xt[k][:, c0:c1],
                                      in_=xt_d[128 * k:128 * (k + 1), c0:c1])

            wv_sb = consts.tile([128, 8, DQ], BF16, tag="wv")
            nc.sync.dma_start(out=wv_sb, in_=wv_d.rearrange("(k p) m -> p k m", p=128))
            wq_sb = consts.tile([128, 8, DQ], BF16, tag="wq")
            nc.sync.dma_start(out=wq_sb, in_=wq_d.rearrange("(k p) m -> p k m", p=128))

            cs_sb = consts.tile([128, 2, S], BF16, tag="cs")
            nc.sync.dma_start(out=cs_sb, in_=cs_d.rearrange("p (a b) -> p a b", a=2))
            cos_sb = cs_sb[:, 0, :]
            sin_sb = cs_sb[:, 1, :]
            ti_sb = consts.tile([128, 2, 128], BF16, tag="ti")
            nc.sync.dma_start(out=ti_sb, in_=ti_d.rearrange("p (a b) -> p a b", a=2))
            mask_sb = ti_sb[:, 0, :]   # 0/1 upper-tri (k <= q) causal mask
            ident_sb = ti_sb[:, 1, :]
            wo_sb = consts.tile([128, 2, D], BF16, tag="wo")
            nc.sync.dma_start(out=wo_sb, in_=wo_d.rearrange("(k p) m -> p k m", p=128))

            # q/k destination tiles: [pair][128 rows = 2 heads x 64, S]
            qt = [qkp.tile([128, S], BF16, tag=f"qt{p}", name=f"qt{p}") for p in range(2)]
            kt = [qkp.tile([128, S], BF16, tag=f"kt{p}", name=f"kt{p}") for p in range(2)]
            # v tiles: per s-chunk [128, 4*65] ([v_h | 1] per head)
            vsb = [vp.tile([128, 4 * 65], BF16, tag=f"v{i}", name=f"v{i}") for i in range(16)]
            # attention output (pre out-proj): [pair][128 = 2 heads x 64 dv, S]
            ot = [qkp.tile([128, S], BF16, tag=f"ot{p}", name=f"ot{p}") for p in range(2)]

            # ---- projection emitters (one unit = one 8-deep matmul group) ----
            def rope_unit(w_sb, dst, n, m, half, on_act):
                hcol = slice(1024 * n + 512 * half, 1024 * n + 512 * half + 512)
                mcol = slice(128 * m, 128 * (m + 1))
                ps = psp.tile([128, 512], F32, tag="ps", name="ps")
                for k in range(8):
                    nc.tensor.matmul(
                        ps,
                        lhsT=w_sb[:, k, mcol],
                        rhs=xt[k][:, hcol],
                        start=(k == 0),
                        stop=(k == 7),
                    )
                # RoPE: dst = raw*cos + shuf(raw)*sin
                raw = ropep.tile([128, 512], BF16, tag="raw")
                (nc.scalar.copy if on_act else nc.vector.tensor_copy)(raw, ps)
                rot = ropep.tile([128, 512], BF16, tag="rot")
                nc.vector.stream_shuffle(rot, raw, SHUF)
                t1 = ropep.tile([128, 512], BF16, tag="rot", name="t1")
                nc.vector.tensor_mul(t1, raw, cos_sb[:, hcol])
                t2 = ropep.tile([128, 512], BF16, tag="t2")
                nc.vector.tensor_mul(t2, rot, sin_sb[:, hcol])
                nc.vector.tensor_add(dst[m][:, hcol], t1, t2)

            def v_unit(n, g, on_act):
                # Two s-chunks share one psum bank as one accumulation group
                # (start on the first chunk's k=0, the second chunk's k=0
                # overwrites its pending-zero half, stop on its k=7).
                psv = psp.tile([128, 2, 512], F32, tag="ps", name="psv")
                for sub in range(4):
                    i = 8 * n + 4 * g + sub
                    scol = slice(128 * i, 128 * (i + 1))
                    half = slice(256 * (sub % 2), 256 * (sub % 2) + 256)
                    for k in range(8):
                        nc.tensor.matmul(
                            psv[:, sub // 2, half],
                            lhsT=xt[k][:, scol],
                            rhs=wv_sb[:, k, :],
                            start=(sub % 2 == 0 and k == 0),
                            stop=(sub % 2 == 1 and k == 7),
                        )
                for sub in range(4):
                    i = 8 * n + 4 * g + sub
                    half = slice(256 * (sub % 2), 256 * (sub % 2) + 256)
                    nc.vector.memset(
                        vsb[i].rearrange("p (h c) -> p h c", c=65)[:, :, 64],
                        1.0,
                    )
                    (nc.scalar.copy if on_act else nc.vector.tensor_copy)(
                        vsb[i].rearrange("p (h c) -> p h c", c=65)[:, :, 0:64],
                        psv[:, sub // 2, half].rearrange("p (h c) -> p h c", c=64),
                    )

            # ---- phase A: k/v/q projections for s-block 0 (pure tensor) ----
            for m in range(2):
                for half in range(2):
                    rope_unit(wk_sb, kt, 0, m, half, on_act=True)
            for g in range(2):
                v_unit(0, g, on_act=True)
            for m in range(2):
                for half in range(2):
                    rope_unit(wq_sb, qt, 0, m, half, on_act=True)
            for k in range(8):
                nc.sync.dma_start(out=xt[k][:, 1024:2048],
                                  in_=xt_d[128 * k:128 * (k + 1), 1024:2048])

            # ---- filler queue: work woven between attention groups so the
            # tensor engine never drains (s-block-1 projections first, then
            # transposes + output projections appended as they become legal).
            filler = []

            def tick(k=1):
                for _ in range(k):
                    if filler:
                        filler.pop(0)()

            for m in range(2):
                for half in range(2):
                    filler.append(
                        lambda m=m, half=half:
                        rope_unit(wk_sb, kt, 1, m, half, on_act=True))
            for g in range(2):
                filler.append(lambda g=g: v_unit(1, g, on_act=True))
            for m in range(2):
                for half in range(2):
                    filler.append(
                        lambda m=m, half=half:
                        rope_unit(wq_sb, qt, 1, m, half, on_act=True))

            def mk_transpose(j, p, otq):
                jcol = slice(512 * j, 512 * (j + 1))

                def go():
                    tp = psp.tile([128, 4, 128], BF16, tag="ps", name="tp")
                    for qc in range(4):
                        nc.tensor.matmul(
                            tp[:, qc, :],
                            lhsT=otq[:, qc, :],
                            rhs=ident_sb,
                            is_transpose=True,
                            start=(qc == 0),
                            stop=(qc == 3),
                            skip_group_check=True,
                        )
                    nc.vector.tensor_copy(ot[p][:, jcol],
                                          tp.rearrange("p a b -> p (a b)"))
                return go

            def mk_po(m):
                def go():
                    mcol = slice(128 * m, 128 * (m + 1))
                    posb = bigp.tile([128, D], BF16, tag="big", name="posb")
                    for d in range(2):
                        po = pop.tile([128, 512], F32, tag="po", name="po")
                        for pp in range(2):
                            nc.tensor.matmul(
                                po,
                                lhsT=ot[pp][:, mcol],
                                rhs=wo_sb[:, pp, 512 * d:512 * (d + 1)],
                                start=(pp == 0),
                                stop=(pp == 1),
                            )
                        eng = nc.vector.tensor_copy if d == 0 else nc.scalar.copy
                        eng(posb[:, 512 * d:512 * (d + 1)], po)
                    nc.sync.dma_start(out=out_d[mcol, :], in_=posb)
                return go

            # ---- phase B: attention, PV lagging its scores by two groups ----
            for j in range(4):
                if j > 0:
                    for m in range(4 * (j - 1), 4 * j):
                        filler.append(mk_po(m))
                for p in range(2):
                    pvq = [
                        pvqp.tile([128, 4, 65], F32, tag=f"pvq{h}",
                                  name=f"pvq{h}")
                        for h in range(2)
                    ]
                    started = [False, False]
                    pvdefer = []

                    def mk_pv(i, probs):
                        r = i - 4 * j

                        def go():
                            for h in range(2):
                                hh = 2 * p + h
                                for qc in range(3, max(r, 0) - 1, -1):
                                    nc.tensor.matmul(
                                        pvq[h][:, qc, :],
                                        lhsT=probs[:, h,
                                                   128 * qc:128 * (qc + 1)],
                                        rhs=vsb[i][:, 65 * hh:65 * hh + 65],
                                        start=(not started[h]),
                                        stop=(i == 4 * j + qc),
                                        skip_group_check=True,
                                    )
                                    started[h] = True
                        return go

                    for i in range(4 * j + 4):
                        r = i - 4 * j
                        loc = max(0, 128 * r)
                        icol = slice(128 * i, 128 * (i + 1))
                        probs = probsp.tile([128, 2, 512], BF16, tag="probs")
                        sc = psp.tile([128, 2, 512], F32, tag="ps", name="sc")
                        for h in range(2):
                            rows = slice(64 * h, 64 * (h + 1))
                            nc.tensor.matmul(
                                sc[:, h, loc:512],
                                lhsT=kt[p][rows, icol],
                                rhs=qt[p][rows, 512 * j + loc:512 * (j + 1)],
                                start=True,
                                stop=True,
                            )
                        nc.scalar.activation(
                            probs[:, :, loc:512], sc[:, :, loc:512], Exp
                        )
                        if r >= 0:
                            for h in range(2):
                                nc.vector.tensor_mul(
                                    probs[:, h, loc:loc + 128],
                                    probs[:, h, loc:loc + 128],
                                    mask_sb,
                                )
                        pvdefer.append(mk_pv(i, probs))
                        if len(pvdefer) > 2:
                            pvdefer.pop(0)()
                        if i % 3 == 0 if j < 2 else True:
                            tick()
                    while pvdefer:
                        pvdefer.pop(0)()
                        tick()
                    # normalization (DVE): otq[:, qc, 64h:64h+64] =
                    #   pvq[h][:, qc, 0:64] / den  (den = column 64)
                    otq = otqp.tile([128, 4, 128], BF16, tag="otq", name="otq")
                    for h in range(2):
                        rd = smallp.tile([128, 4], F32, tag="rd", name="rd")
                        nc.vector.reciprocal(rd, pvq[h][:, :, 64])
                        for qc in range(4):
                            nc.vector.tensor_scalar_mul(
                                otq[:, qc, 64 * h:64 * (h + 1)],
                                pvq[h][:, qc, 0:64],
                                rd[:, qc:qc + 1],
                            )
                    filler.append(mk_transpose(j, p, otq))
            while filler:
                tick()
            for m in range(12, 16):
                mk_po(m)()

    nc.compile()
    return nc


def _host_inputs(x, Wq, Wk, Wv, Wo, token_positions):
    """Build per-core input maps (all host-side numpy prep)."""
    bf = ml_dtypes.bfloat16
    x = np.asarray(x, dtype=np.float32)
    Wq = np.asarray(Wq, dtype=np.float32)
    Wk = np.asarray(Wk, dtype=np.float32)
    Wv = np.asarray(Wv, dtype=np.float32)
    Wo = np.asarray(Wo, dtype=np.float32)
    pos = np.asarray(token_positions).astype(np.float64)

    # RoPE tables in the permuted-lane layout (16-lane e/o blocks).
    idx = np.arange(0, HD, 2, dtype=np.float64) / HD
    freqs = 1.0 / THETA ** idx                      # [32]
    ang = pos[:, None] * freqs[None, :]             # [S, 32]
    c, s = np.cos(ang).T, np.sin(ang).T             # [32, S]
    c64 = np.concatenate([c[0:16], c[0:16], c[16:32], c[16:32]], 0)
    s64 = np.concatenate([-s[0:16], s[0:16], -s[16:32], s[16:32]], 0)
    cos128 = np.concatenate([c64, c64], 0).astype(np.float32)
    sin128 = np.concatenate([s64, s64], 0).astype(np.float32)
    cs128 = np.concatenate([cos128, sin128], 1).astype(bf)  # [128, 2S]

    # 0/1 upper-triangular causal mask (valid iff k <= q) + identity
    mask01 = np.triu(np.ones((128, 128), dtype=np.float32))
    ident = np.eye(128, dtype=np.float32)
    ti = np.concatenate([mask01, ident], 1).astype(bf)      # [128, 256]

    # per-head row permutation: [e0..e15, o0..o15, e16..e31, o16..o31]
    perm64 = np.concatenate([
        np.arange(0, 32, 2), np.arange(1, 32, 2),
        np.arange(32, 64, 2), np.arange(33, 64, 2),
    ])

    xts = [np.ascontiguousarray(x[b].T).astype(bf) for b in range(B)]

    in_maps = []
    for core in range(NCORES):
        b = core // 4
        heads = [4 * (core % 4) + hh for hh in range(HPC)]
        qk_rows = np.concatenate([g * HD + perm64 for g in heads])
        v_rows = np.concatenate([np.arange(g * HD, (g + 1) * HD) for g in heads])
        in_maps.append({
            "xt": xts[b],
            "wq": (np.ascontiguousarray(Wq[qk_rows, :].T) / np.sqrt(HD)).astype(bf),
            "wk": np.ascontiguousarray(Wk[qk_rows, :].T).astype(bf),
            "wv": np.ascontiguousarray(Wv[v_rows, :].T).astype(bf),
            "wo": np.ascontiguousarray(Wo[:, v_rows].T).astype(bf),
            "cs": cs128,
            "ti": ti,
        })
    return in_maps


def _ensure_ntff_hook():
    """Register the axon NTFF profile hook if the image's antenv lacks it."""
    import sys, types
    try:
        import antenv.axon_hooks  # noqa: F401
        return
    except ImportError:
        pass
    try:
        from trn_agent_boot.trn_boot import _ntff_profile_via_ctypes
        hook = _ntff_profile_via_ctypes("/opt/axon/libaxon_pjrt.so")
    except Exception:
        return
    mod = types.ModuleType("antenv.axon_hooks")
    mod.get_axon_ntff_profile_hook = lambda: hook
    mod.set_axon_ntff_profile_hook = lambda h: None
    sys.modules["antenv.axon_hooks"] = mod


def run(inputs, trace=False):
    """Run the SPMD kernel; returns (full_output, BassKernelResults)."""
    if trace:
        _ensure_ntff_hook()
    if "nc" not in _cached:
        _cached["nc"] = build_nc()
    nc = _cached["nc"]
    in_maps = _host_inputs(
        inputs["x"], inputs["Wq"], inputs["Wk"], inputs["Wv"], inputs["Wo"],
        inputs["token_positions"],
    )
    res = run_bass_kernel_spmd(nc, in_maps, core_ids=list(range(NCORES)),
                               trace=trace)
    out = np.zeros((B, S, D), dtype=np.float32)
    for core in range(NCORES):
        out[core // 4] += res.results[core]["out"].astype(np.float32)
    return out, res


def kernel(**inputs) -> np.ndarray:
    out, _ = run(inputs, trace=False)
    return out


# revision 27
# speedup vs baseline: 1.0769x; 1.0457x over previous
"""Multi-head attention (RoPE, causal) Bass kernel for 8 TRN2 NeuronCores.

Problem: x[2,2048,1024], 16 heads x 64 dim, causal mask, RoPE, f32.

Sharding: batch x head-group. Core c handles batch c//4 and the 4 heads
[4*(c%4), 4*(c%4)+4). Each core computes q/k/v projections for its head
slice, RoPE, causal attention, and a partial output projection against its
rows of Wo.T. The host sums the 4 partials per batch (the "all-reduce" of
the row-split output projection is done on the host during unsharding).

Device layout notes:
- All inputs stream in bf16 (f32 PSUM accumulation everywhere); the output
  partials return in bf16 and are upcast + summed on the host.
- x is passed pre-transposed per batch: xT [1024, 2048] so it can stream as
  the matmul moving operand.
- Wq/Wk rows are permuted per head to [e0..e15, o0..o15, e16..e31, o16..o31]
  (e/o = even/odd RoPE pair lanes) so the RoPE rotate-half becomes a
  16<->16 swap inside each 32-partition group (one DVE stream_shuffle).
- Causal masking of the diagonal 128-blocks is a post-exp DVE multiply with
  a 0/1 upper-triangular tile (cheaper than a -inf accumulate matmul).
- PV runs q-major: out[q,65] = probs[k,q].T @ [v|1][k,65] accumulated over
  k-chunks. The appended ones column makes the softmax denominator land in
  column 64 -- one value per PSUM partition -- so normalization is a cheap
  strided reciprocal + per-partition multiply. The normalized [q, dv] tiles
  are transposed back to [dv, q] on the PE for the output projection.
- Scheduling: the s-block-1 projections, PE transposes and output
  projection are emitted as filler closures woven between attention groups,
  and PV matmuls trail their scores by two groups, so the tensor queue
  always holds ready work and never idles waiting on the scalar engine's
  EXP (the PE p-state drops ~2x for 3us after any idle gap).
"""

import numpy as np
import ml_dtypes

import concourse.bass as bass
import concourse.mybir as mybir
import concourse.tile as tile
from concourse import bacc
from concourse.bass_utils import run_bass_kernel_spmd

F32 = mybir.dt.float32
BF16 = mybir.dt.bfloat16

B, S, D = 2, 2048, 1024
H, HD = 16, 64
NCORES = 8
HPC = 4          # heads per core
DQ = HPC * HD    # 256 projected dims per core
THETA = 10000.0

_cached = {}


def build_nc():
    """Build the single-core Bass graph (same NEFF runs SPMD on all 8)."""
    nc = bacc.Bacc("TRN2", target_bir_lowering=False)

    xt_d = nc.dram_tensor("xt", [D, S], BF16, kind="ExternalInput")
    wq_d = nc.dram_tensor("wq", [D, DQ], BF16, kind="ExternalInput")
    wk_d = nc.dram_tensor("wk", [D, DQ], BF16, kind="ExternalInput")
    wv_d = nc.dram_tensor("wv", [D, DQ], BF16, kind="ExternalInput")
    wo_d = nc.dram_tensor("wo", [DQ, D], BF16, kind="ExternalInput")
    cs_d = nc.dram_tensor("cs", [128, 2 * S], BF16, kind="ExternalInput")
    ti_d = nc.dram_tensor("ti", [128, 256], BF16, kind="ExternalInput")
    out_d = nc.dram_tensor("out", [S, D], BF16, kind="ExternalOutput")

    Exp = mybir.ActivationFunctionType.Exp
    SHUF = [(i + 16) % 32 for i in range(32)]  # 16<->16 swap per 32-group

    with tile.TileContext(nc) as tc:
        with (
            tc.tile_pool(name="consts", bufs=1) as consts,
            tc.tile_pool(name="big", bufs=8) as bigp,
            tc.tile_pool(name="qk", bufs=1) as qkp,
            tc.tile_pool(name="vsb", bufs=1) as vp,
            tc.tile_pool(name="rope", bufs=4) as ropep,
            tc.tile_pool(name="probs", bufs=4) as probsp,
            tc.tile_pool(name="otq", bufs=2) as otqp,
            tc.tile_pool(name="small", bufs=3) as smallp,
            tc.tile_pool(name="ps", bufs=2, space="PSUM") as psp,
            tc.tile_pool(name="pvq", bufs=1, space="PSUM") as pvqp,
            tc.tile_pool(name="po", bufs=2, space="PSUM") as pop,
        ):
            # ---- weights + xT (staged: first 512 cols ahead, k-proj first) ----
            wk_sb = consts.tile([128, 8, DQ], BF16, tag="wk")
            nc.sync.dma_start(out=wk_sb, in_=wk_d.rearrange("(k p) m -> p k m", p=128))

            xt = []
            for k in range(8):
                t = bigp.tile([128, S], BF16, tag="big", name=f"xt{k}")
                xt.append(t)
            for c0, c1 in ((0, 512), (512, 1024)):
                for k in range(8):
                    nc.sync.dma_start(out=xt[k][:, c0:c1],
                                      in_=xt_d[128 * k:128 * (k + 1), c0:c1])

            wv_sb = consts.tile([128, 8, DQ], BF16, tag="wv")
            nc.sync.dma_start(out=wv_sb, in_=wv_d.rearrange("(k p) m -> p k m", p=128))
            wq_sb = consts.tile([128, 8, DQ], BF16, tag="wq")
            nc.sync.dma_start(out=wq_sb, in_=wq_d.rearrange("(k p) m -> p k m", p=128))

            cs_sb = consts.tile([128, 2, S], BF16, tag="cs")
            nc.sync.dma_start(out=cs_sb, in_=cs_d.rearrange("p (a b) -> p a b", a=2))
            cos_sb = cs_sb[:, 0, :]
            sin_sb = cs_sb[:, 1, :]
            ti_sb = consts.tile([128, 2, 128], BF16, tag="ti")
            nc.sync.dma_start(out=ti_sb, in_=ti_d.rearrange("p (a b) -> p a b", a=2))
            mask_sb = ti_sb[:, 0, :]   # 0/1 upper-tri (k <= q) causal mask
            ident_sb = ti_sb[:, 1, :]
            wo_sb = consts.tile([128, 2, D], BF16, tag="wo")
            nc.sync.dma_start(out=wo_sb, in_=wo_d.rearrange("(k p) m -> p k m", p=128))

            # q/k destination tiles: [pair][128 rows = 2 heads x 64, S]
            qt = [qkp.tile([128, S], BF16, tag=f"qt{p}", name=f"qt{p}") for p in range(2)]
            kt = [qkp.tile([128, S], BF16, tag=f"kt{p}", name=f"kt{p}") for p in range(2)]
            # v tiles: per s-chunk [128, 4*65] ([v_h | 1] per head)
            vsb = [vp.tile([128, 4 * 65], BF16, tag=f"v{i}", name=f"v{i}") for i in range(16)]
            # attention output (pre out-proj): [pair][128 = 2 heads x 64 dv, S]
            ot = [qkp.tile([128, S], BF16, tag=f"ot{p}", name=f"ot{p}") for p in range(2)]

            # ---- projection emitters (one unit = one 8-deep matmul group) ----
            def rope_unit(w_sb, dst, n, m, half, on_act):
                hcol = slice(1024 * n + 512 * half, 1024 * n + 512 * half + 512)
                mcol = slice(128 * m, 128 * (m + 1))
                ps = psp.tile([128, 512], F32, tag="ps", name="ps")
                for k in range(8):
                    nc.tensor.matmul(
                        ps,
                        lhsT=w_sb[:, k, mcol],
                        rhs=xt[k][:, hcol],
                        start=(k == 0),
                        stop=(k == 7),
                    )
                # RoPE: dst = raw*cos + shuf(raw)*sin
                raw = ropep.tile([128, 512], BF16, tag="raw")
                (nc.scalar.copy if on_act else nc.vector.tensor_copy)(raw, ps)
                rot = ropep.tile([128, 512], BF16, tag="rot")
                nc.vector.stream_shuffle(rot, raw, SHUF)
                t1 = ropep.tile([128, 512], BF16, tag="rot", name="t1")
                nc.vector.tensor_mul(t1, raw, cos_sb[:, hcol])
                t2 = ropep.tile([128, 512], BF16, tag="t2")
                nc.vector.tensor_mul(t2, rot, sin_sb[:, hcol])
                nc.vector.tensor_add(dst[m][:, hcol], t1, t2)

            def v_unit(n, g, on_act):
                # Two s-chunks share one psum bank as one accumulation group
                # (start on the first chunk's k=0, the second chunk's k=0
                # overwrites its pending-zero half, stop on its k=7).
                psv = psp.tile([128, 2, 512], F32, tag="ps", name="psv")
                for sub in range(4):
                    i = 8 * n + 4 * g + sub
                    scol = slice(128 * i, 128 * (i + 1))
                    half = slice(256 * (sub % 2), 256 * (sub % 2) + 256)
                    for k in range(8):
                        nc.tensor.matmul(
                            psv[:, sub // 2, half],
                            lhsT=xt[k][:, scol],
                            rhs=wv_sb[:, k, :],
                            start=(sub % 2 == 0 and k == 0),
                            stop=(sub % 2 == 1 and k == 7),
                        )
                for sub in range(4):
                    i = 8 * n + 4 * g + sub
                    half = slice(256 * (sub % 2), 256 * (sub % 2) + 256)
                    nc.vector.memset(
                        vsb[i].rearrange("p (h c) -> p h c", c=65)[:, :, 64],
                        1.0,
                    )
                    (nc.scalar.copy if on_act else nc.vector.tensor_copy)(
                        vsb[i].rearrange("p (h c) -> p h c", c=65)[:, :, 0:64],
                        psv[:, sub // 2, half].rearrange("p (h c) -> p h c", c=64),
                    )

            # ---- phase A: k/v/q projections for s-block 0 (pure tensor) ----
            for m in range(2):
                for half in range(2):
                    rope_unit(wk_sb, kt, 0, m, half, on_act=True)
            for g in range(2):
                v_unit(0, g, on_act=True)
            for m in range(2):
                for half in range(2):
                    rope_unit(wq_sb, qt, 0, m, half, on_act=True)
            for k in range(8):
                nc.sync.dma_start(out=xt[k][:, 1024:2048],
                                  in_=xt_d[128 * k:128 * (k + 1), 1024:2048])

            # ---- filler queue: work woven between attention groups so the
            # tensor engine never drains (s-block-1 projections first, then
            # transposes + output projections appended as they become legal).
            filler = []

            def tick(k=1):
                for _ in range(k):
                    if filler:
                        filler.pop(0)()

            for m in range(2):
                for half in range(2):
                    filler.append(
                        lambda m=m, half=half:
                        rope_unit(wk_sb, kt, 1, m, half, on_act=True))
            for g in range(2):
                filler.append(lambda g=g: v_unit(1, g, on_act=True))
            for m in range(2):
                for half in range(2):
                    filler.append(
                        lambda m=m, half=half:
                        rope_unit(wq_sb, qt, 1, m, half, on_act=True))

            def mk_transpose(j, p, otq):
                jcol = slice(512 * j, 512 * (j + 1))

                def go():
                    tp = psp.tile([128, 4, 128], BF16, tag="ps", name="tp")
                    for qc in range(4):
                        nc.tensor.matmul(
                            tp[:, qc, :],
                            lhsT=otq[:, qc, :],
                            rhs=ident_sb,
                            is_transpose=True,
                            start=(qc == 0),
                            stop=(qc == 3),
                            skip_group_check=True,
                        )
                    nc.vector.tensor_copy(ot[p][:, jcol],
                                          tp.rearrange("p a b -> p (a b)"))
                return go

            def mk_po(m):
                def go():
                    mcol = slice(128 * m, 128 * (m + 1))
                    posb = bigp.tile([128, D], BF16, tag="big", name="posb")
                    for d in range(2):
                        po = pop.tile([128, 512], F32, tag="po", name="po")
                        for pp in range(2):
                            nc.tensor.matmul(
                                po,
                                lhsT=ot[pp][:, mcol],
                                rhs=wo_sb[:, pp, 512 * d:512 * (d + 1)],
                                start=(pp == 0),
                                stop=(pp == 1),
                            )
                        eng = nc.vector.tensor_copy if d == 0 else nc.scalar.copy
                        eng(posb[:, 512 * d:512 * (d + 1)], po)
                    nc.sync.dma_start(out=out_d[mcol, :], in_=posb)
                return go

            # ---- phase B: attention, PV lagging its scores by two groups ----
            for j in range(4):
                for p in range(2):
                    pvq = [
                        pvqp.tile([128, 4, 65], F32, tag=f"pvq{h}",
                                  name=f"pvq{h}")
                        for h in range(2)
                    ]
                    started = [False, False]
                    pvdefer = []

                    def mk_pv(i, probs):
                        r = i - 4 * j

                        def go():
                            for h in range(2):
                                hh = 2 * p + h
                                for qc in range(3, max(r, 0) - 1, -1):
                                    nc.tensor.matmul(
                                        pvq[h][:, qc, :],
                                        lhsT=probs[:, h,
                                                   128 * qc:128 * (qc + 1)],
                                        rhs=vsb[i][:, 65 * hh:65 * hh + 65],
                                        start=(not started[h]),
                                        stop=(i == 4 * j + qc),
                                        skip_group_check=True,
                                    )
                                    started[h] = True
                        return go

                    for i in range(4 * j + 4):
                        r = i - 4 * j
                        loc = max(0, 128 * r)
                        icol = slice(128 * i, 128 * (i + 1))
                        probs = probsp.tile([128, 2, 512], BF16, tag="probs")
                        sc = psp.tile([128, 2, 512], F32, tag="ps", name="sc")
                        for h in range(2):
                            rows = slice(64 * h, 64 * (h + 1))
                            nc.tensor.matmul(
                                sc[:, h, loc:512],
                                lhsT=kt[p][rows, icol],
                                rhs=qt[p][rows, 512 * j + loc:512 * (j + 1)],
                                start=True,
                                stop=True,
                            )
                        nc.scalar.activation(
                            probs[:, :, loc:512], sc[:, :, loc:512], Exp
                        )
                        if r >= 0:
                            for h in range(2):
                                nc.vector.tensor_mul(
                                    probs[:, h, loc:loc + 128],
                                    probs[:, h, loc:loc + 128],
                                    mask_sb,
                                )
                        pvdefer.append(mk_pv(i, probs))
                        if len(pvdefer) > 2:
                            pvdefer.pop(0)()
                        if i % 3 == 0 if j < 2 else True:
                            tick()
                    while pvdefer:
                        pvdefer.pop(0)()
                        tick()
                    # normalization (DVE): otq[:, qc, 64h:64h+64] =
                    #   pvq[h][:, qc, 0:64] / den  (den = column 64)
                    otq = otqp.tile([128, 4, 128], BF16, tag="otq", name="otq")
                    for h in range(2):
                        rd = smallp.tile([128, 4], F32, tag="rd", name="rd")
                        nc.vector.reciprocal(rd, pvq[h][:, :, 64])
                        for qc in range(4):
                            nc.vector.tensor_scalar_mul(
                                otq[:, qc, 64 * h:64 * (h + 1)],
                                pvq[h][:, qc, 0:64],
                                rd[:, qc:qc + 1],
                            )
                    filler.append(mk_transpose(j, p, otq))
                if j > 0:
                    for m in range(4 * (j - 1), 4 * j):
                        filler.append(mk_po(m))
            while filler:
                tick()
            for m in range(12, 16):
                mk_po(m)()

    nc.compile()
    return nc


def _host_inputs(x, Wq, Wk, Wv, Wo, token_positions):
    """Build per-core input maps (all host-side numpy prep)."""
    bf = ml_dtypes.bfloat16
    x = np.asarray(x, dtype=np.float32)
    Wq = np.asarray(Wq, dtype=np.float32)
    Wk = np.asarray(Wk, dtype=np.float32)
    Wv = np.asarray(Wv, dtype=np.float32)
    Wo = np.asarray(Wo, dtype=np.float32)
    pos = np.asarray(token_positions).astype(np.float64)

    # RoPE tables in the permuted-lane layout (16-lane e/o blocks).
    idx = np.arange(0, HD, 2, dtype=np.float64) / HD
    freqs = 1.0 / THETA ** idx                      # [32]
    ang = pos[:, None] * freqs[None, :]             # [S, 32]
    c, s = np.cos(ang).T, np.sin(ang).T             # [32, S]
    c64 = np.concatenate([c[0:16], c[0:16], c[16:32], c[16:32]], 0)
    s64 = np.concatenate([-s[0:16], s[0:16], -s[16:32], s[16:32]], 0)
    cos128 = np.concatenate([c64, c64], 0).astype(np.float32)
    sin128 = np.concatenate([s64, s64], 0).astype(np.float32)
    cs128 = np.concatenate([cos128, sin128], 1).astype(bf)  # [128, 2S]

    # 0/1 upper-triangular causal mask (valid iff k <= q) + identity
    mask01 = np.triu(np.ones((128, 128), dtype=np.float32))
    ident = np.eye(128, dtype=np.float32)
    ti = np.concatenate([mask01, ident], 1).astype(bf)      # [128, 256]

    # per-head row permutation: [e0..e15, o0..o15, e16..e31, o16..o31]
    perm64 = np.concatenate([
        np.arange(0, 32, 2), np.arange(1, 32, 2),
        np.arange(32, 64, 2), np.arange(33, 64, 2),
    ])

    xts = [np.ascontiguousarray(x[b].T).astype(bf) for b in range(B)]

    in_maps = []
    for core in range(NCORES):
        b = core // 4
        heads = [4 * (core % 4) + hh for hh in range(HPC)]
        qk_rows = np.concatenate([g * HD + perm64 for g in heads])
        v_rows = np.concatenate([np.arange(g * HD, (g + 1) * HD) for g in heads])
        in_maps.append({
            "xt": xts[b],
            "wq": (np.ascontiguousarray(Wq[qk_rows, :].T) / np.sqrt(HD)).astype(bf),
            "wk": np.ascontiguousarray(Wk[qk_rows, :].T).astype(bf),
            "wv": np.ascontiguousarray(Wv[v_rows, :].T).astype(bf),
            "wo": np.ascontiguousarray(Wo[:, v_rows].T).astype(bf),
            "cs": cs128,
            "ti": ti,
        })
    return in_maps


def _ensure_ntff_hook():
    """Register the axon NTFF profile hook if the image's antenv lacks it."""
    import sys, types
    try:
        import antenv.axon_hooks  # noqa: F401
        return
    except ImportError:
        pass
    try:
        from trn_agent_boot.trn_boot import _ntff_profile_via_ctypes
        hook = _ntff_profile_via_ctypes("/opt/axon/libaxon_pjrt.so")
    except Exception:
        return
    mod = types.ModuleType("antenv.axon_hooks")
    mod.get_axon_ntff_profile_hook = lambda: hook
    mod.set_axon_ntff_profile_hook = lambda h: None
    sys.modules["antenv.axon_hooks"] = mod


def run(inputs, trace=False):
    """Run the SPMD kernel; returns (full_output, BassKernelResults)."""
    if trace:
        _ensure_ntff_hook()
    if "nc" not in _cached:
        _cached["nc"] = build_nc()
    nc = _cached["nc"]
    in_maps = _host_inputs(
        inputs["x"], inputs["Wq"], inputs["Wk"], inputs["Wv"], inputs["Wo"],
        inputs["token_positions"],
    )
    res = run_bass_kernel_spmd(nc, in_maps, core_ids=list(range(NCORES)),
                               trace=trace)
    out = np.zeros((B, S, D), dtype=np.float32)
    for core in range(NCORES):
        out[core // 4] += res.results[core]["out"].astype(np.float32)
    return out, res


def kernel(**inputs) -> np.ndarray:
    out, _ = run(inputs, trace=False)
    return out
